# revision 2
# baseline (speedup 1.0000x reference)
"""MoE expert-parallel kernel for Trainium2 (8 NeuronCores).

Computation (full shapes):
  inputs  [E*CAP, D_IN] = [16384, 4096] f32   (expert-major dispatched tokens)
  weight  [E, D_IN, D_OUT] = [8, 4096, 1024]
  bias    [E, 1, D_OUT]
  cw      [S, E, CAP] = [8192, 8, 2048]
  y    = inputs.reshape(E,CAP,D_IN) @ weight + bias          (grouped GEMM)
  out  = cw.reshape(S, E*CAP) @ y.reshape(E*CAP, D_OUT)      (combine)

Sharding (8 cores):
  Stage 1: expert parallel — core c computes y_c = x_c @ W_c + b_c  [2048,1024]
  AllGather(bf16 y) -> flat [16384, 1024] on every core
  Stage 2: token parallel — core c computes out rows [c*1024, (c+1)*1024)

All matmul inputs are bf16 (cast during SWDGE DMA), fp32 PSUM accumulation,
fp32 SBUF accumulator across k-chunks in stage 2.
"""

import numpy as np

E, CAP, D_IN, D_OUT = 8, 2048, 4096, 1024
S = 8192
N_CORES = 8
TOK = S // N_CORES  # 1024 tokens per core in stage 2

_CACHE = {}


def _build_nc():
    import concourse.mybir as mybir
    import concourse.tile as tile
    from concourse import bacc

    f32 = mybir.dt.float32
    bf16 = mybir.dt.bfloat16

    nc = bacc.Bacc(
        "TRN2", target_bir_lowering=False, debug=False, num_devices=N_CORES
    )

    x_in = nc.dram_tensor("x", [CAP, D_IN], f32, kind="ExternalInput")
    w_in = nc.dram_tensor("w", [D_IN, D_OUT], f32, kind="ExternalInput")
    b_in = nc.dram_tensor("b", [1, D_OUT], f32, kind="ExternalInput")
    cw_in = nc.dram_tensor("cw", [TOK, E * CAP], f32, kind="ExternalInput")
    out_t = nc.dram_tensor("out", [TOK, D_OUT], f32, kind="ExternalOutput")

    y_dram = nc.dram_tensor("y_local", [CAP, D_OUT], bf16)
    flat_dram = nc.dram_tensor("flat", [E * CAP, D_OUT], bf16, addr_space="Shared")

    KT1 = D_IN // 128  # 32 contraction tiles, stage 1
    TT1 = CAP // 128  # 16 token tiles, stage 1
    CH = 8  # k-chunks in stage 2 (one per expert)
    KT2 = (E * CAP) // CH // 128  # 16 contraction tiles per chunk
    TT2 = TOK // 128  # 8 token tiles, stage 2

    with tile.TileContext(nc) as tc:
        with tc.tile_pool(name="const", bufs=1) as cpool:
            ones_sb = cpool.tile([1, 128], bf16)
            nc.vector.memset(ones_sb[:], 1.0)
            bias_sb = cpool.tile([1, D_OUT], bf16)
            nc.gpsimd.dma_start(out=bias_sb[:], in_=b_in.ap())

            # ---------------- stage 1: y_c = x_c @ W_c + b_c ----------------
            with (
                tc.tile_pool(name="wpool", bufs=1) as wp,
                tc.tile_pool(name="s1", bufs=2) as s1,
                tc.tile_pool(name="ps1", bufs=4, space="PSUM") as pp1,
            ):
                w_sb = wp.tile([128, KT1, D_OUT], bf16)
                nc.gpsimd.dma_start(
                    out=w_sb[:],
                    in_=w_in.ap().rearrange("(kt p) n -> p kt n", p=128),
                )
                for tt in range(TT1):
                    x_ld = s1.tile([128, D_IN], bf16, tag="xld")
                    nc.gpsimd.dma_start(
                        out=x_ld[:], in_=x_in.ap()[tt * 128 : (tt + 1) * 128, :]
                    )
                    xT = s1.tile([128, KT1, 128], bf16, tag="xT")
                    nc.sync.dma_start_transpose(xT[:], x_ld[:])
                    y_sb = s1.tile([128, D_OUT], bf16, tag="ysb")
                    for dh in range(2):
                        ps = pp1.tile([128, 512], f32)
                        dsl = slice(dh * 512, (dh + 1) * 512)
                        for kt in range(KT1):
                            nc.tensor.matmul(
                                ps[:],
                                lhsT=xT[:, kt, :],
                                rhs=w_sb[:, kt, dsl],
                                start=(kt == 0),
                                stop=False,
                            )
                        # bias via rank-1 update: ones[1,128].T @ b[1,512]
                        nc.tensor.matmul(
                            ps[:],
                            lhsT=ones_sb[:1, :],
                            rhs=bias_sb[:1, dsl],
                            start=False,
                            stop=True,
                        )
                        nc.vector.tensor_copy(out=y_sb[:, dsl], in_=ps[:])
                    nc.sync.dma_start(
                        out=y_dram[tt * 128 : (tt + 1) * 128, :], in_=y_sb[:]
                    )

            # ---------------- all-gather expert outputs ----------------
            nc.gpsimd.collective_compute(
                "AllGather",
                mybir.AluOpType.bypass,
                replica_groups=[list(range(N_CORES))],
                ins=[y_dram.ap().opt()],
                outs=[flat_dram.ap().opt()],
            )

            # ---------------- stage 2: out = cw_c @ flat ----------------
            with (
                tc.tile_pool(name="s2", bufs=2) as s2,
                tc.tile_pool(name="accp", bufs=1) as accp,
                tc.tile_pool(name="ps2", bufs=4, space="PSUM") as pp2,
            ):
                acc = accp.tile([128, TT2, D_OUT], f32)
                for ch in range(CH):
                    flat_sb = s2.tile([128, KT2, D_OUT], bf16, tag="flat")
                    nc.sync.dma_start(
                        out=flat_sb[:],
                        in_=flat_dram.ap()[
                            ch * 2048 : (ch + 1) * 2048, :
                        ].rearrange("(kt p) n -> p kt n", p=128),
                    )
                    for t2 in range(TT2):
                        cw_ld = s2.tile([128, 2048], bf16, tag="cwld")
                        nc.gpsimd.dma_start(
                            out=cw_ld[:],
                            in_=cw_in.ap()[
                                t2 * 128 : (t2 + 1) * 128,
                                ch * 2048 : (ch + 1) * 2048,
                            ],
                        )
                        cwT = s2.tile([128, KT2, 128], bf16, tag="cwT")
                        nc.sync.dma_start_transpose(cwT[:], cw_ld[:])
                        for dh in range(2):
                            ps2 = pp2.tile([128, 512], f32)
                            dsl = slice(dh * 512, (dh + 1) * 512)
                            for kt in range(KT2):
                                nc.tensor.matmul(
                                    ps2[:],
                                    lhsT=cwT[:, kt, :],
                                    rhs=flat_sb[:, kt, dsl],
                                    start=(kt == 0),
                                    stop=(kt == KT2 - 1),
                                )
                            if ch == 0:
                                nc.vector.tensor_copy(
                                    out=acc[:, t2, dsl], in_=ps2[:]
                                )
                            else:
                                nc.vector.tensor_add(
                                    acc[:, t2, dsl], acc[:, t2, dsl], ps2[:]
                                )
                for t2 in range(TT2):
                    nc.sync.dma_start(
                        out=out_t[t2 * 128 : (t2 + 1) * 128, :],
                        in_=acc[:, t2, :],
                    )

    nc.compile()
    return nc


def _get_nc():
    if "nc" not in _CACHE:
        _CACHE["nc"] = _build_nc()
    return _CACHE["nc"]


def make_in_maps(inputs, combine_weights, weight, bias):
    inputs = np.ascontiguousarray(np.asarray(inputs, dtype=np.float32))
    combine_weights = np.ascontiguousarray(
        np.asarray(combine_weights, dtype=np.float32)
    )
    weight = np.ascontiguousarray(np.asarray(weight, dtype=np.float32))
    bias = np.ascontiguousarray(np.asarray(bias, dtype=np.float32))

    in_maps = []
    for c in range(N_CORES):
        in_maps.append(
            {
                "x": inputs[c * CAP : (c + 1) * CAP],
                "w": weight[c],
                "b": bias[c].reshape(1, D_OUT),
                "cw": combine_weights[c * TOK : (c + 1) * TOK].reshape(
                    TOK, E * CAP
                ),
            }
        )
    return in_maps


def kernel(inputs, combine_weights, weight, bias):
    from concourse.bass_utils import run_bass_kernel_spmd

    nc = _get_nc()
    in_maps = make_in_maps(inputs, combine_weights, weight, bias)
    res = run_bass_kernel_spmd(nc, in_maps, core_ids=list(range(N_CORES)))
    _CACHE["last_results"] = res

    out = np.concatenate([r["out"] for r in res.results], axis=0)
    return out.reshape(4, 2048, D_OUT).astype(np.float32, copy=False)


# revision 13
# speedup vs baseline: 1.1383x; 1.1383x over previous
"""MoE expert-parallel kernel for Trainium2 (8 NeuronCores).

Computation (full shapes):
  inputs  [E*CAP, D_IN] = [16384, 4096] f32   (expert-major dispatched tokens)
  weight  [E, D_IN, D_OUT] = [8, 4096, 1024]
  bias    [E, 1, D_OUT]
  cw      [S, E, CAP] = [8192, 8, 2048]
  y    = inputs.reshape(E,CAP,D_IN) @ weight + bias          (grouped GEMM)
  out  = cw.reshape(S, E*CAP) @ y.reshape(E*CAP, D_OUT)      (combine)

Sharding (8 cores):
  Stage 1: expert parallel — core c computes y_c = x_c @ W_c + b_c  [2048,1024]
  AllGather(bf16 y, 2 cap-half chunks overlapped with stage-1 tail)
  Stage 2: token parallel — core c computes out rows [c*1024, (c+1)*1024)

All matmul inputs are bf16 (cast during SWDGE DMA), fp32 PSUM accumulation,
fp32 SBUF accumulator across k-chunks in stage 2.

flat row order after the chunked AllGather: flat[j] rows = [expert e][cap'],
cap' in [j*1024, (j+1)*1024); stage-2 k-chunk (j, e) therefore reads cw
columns e*2048 + j*1024 + [0, 1024).
"""

import numpy as np

E, CAP, D_IN, D_OUT = 8, 2048, 4096, 1024
S = 8192
N_CORES = 8
TOK = S // N_CORES  # 1024 tokens per core in stage 2

_CACHE = {}


def _build_nc():
    import concourse.mybir as mybir
    import concourse.tile as tile
    from concourse import bacc

    f32 = mybir.dt.float32
    bf16 = mybir.dt.bfloat16

    nc = bacc.Bacc(
        "TRN2", target_bir_lowering=False, debug=False, num_devices=N_CORES
    )

    x_in = nc.dram_tensor("x", [CAP, D_IN], f32, kind="ExternalInput")
    w_in = nc.dram_tensor("w", [D_IN, D_OUT], f32, kind="ExternalInput")
    b_in = nc.dram_tensor("b", [1, D_OUT], f32, kind="ExternalInput")
    cw_in = nc.dram_tensor("cw", [TOK, E * CAP], f32, kind="ExternalInput")
    out_t = nc.dram_tensor("out", [TOK, D_OUT], f32, kind="ExternalOutput")

    HALF = CAP // 2  # 1024 caps per AG chunk
    y_dram = [
        nc.dram_tensor(f"y_local{j}", [HALF, D_OUT], bf16) for j in range(2)
    ]
    flat_dram = [
        nc.dram_tensor(f"flat{j}", [N_CORES * HALF, D_OUT], bf16, addr_space="Shared")
        for j in range(2)
    ]

    KT1 = D_IN // 128  # 32 contraction tiles, stage 1
    TT1 = CAP // 128  # 16 token tiles, stage 1
    NHC = 16  # stage-2 chunks: (j, e) pairs, 1024 k each
    KT2 = 8  # contraction tiles per chunk
    TT2 = TOK // 128  # 8 token tiles, stage 2

    with tile.TileContext(nc) as tc:
        with (
            tc.tile_pool(name="const", bufs=1) as cpool,
            tc.tile_pool(name="flatp", bufs=3) as flatp,
            tc.tile_pool(name="s2", bufs=2) as s2,
        ):
            ones_sb = cpool.tile([1, 128], bf16)
            nc.vector.memset(ones_sb[:], 1.0)
            bias_sb = cpool.tile([1, D_OUT], bf16)
            nc.gpsimd.dma_start(out=bias_sb[:], in_=b_in.ap())

            # stage-2 chunk loads (flat rhs + combine-weight block), issued
            # up to 2 chunks ahead of consumption
            pend = {}

            def load_chunk(hc):
                j, e = hc // 8, hc % 8
                flat_sb = flatp.tile([128, KT2, D_OUT], bf16, tag="flat")
                nc.sync.dma_start(
                    out=flat_sb[:],
                    in_=flat_dram[j]
                    .ap()[e * HALF : (e + 1) * HALF, :]
                    .rearrange("(kt p) n -> p kt n", p=128),
                )
                cw_big = s2.tile([128, TT2, HALF], bf16, tag="cwbig")
                nc.gpsimd.dma_start(
                    out=cw_big[:],
                    in_=cw_in.ap()[
                        :, e * CAP + j * HALF : e * CAP + (j + 1) * HALF
                    ].rearrange("(t p) k -> p t k", p=128),
                )
                pend[hc] = (flat_sb, cw_big)

            # ---------------- stage 1 ----------------
            with (
                tc.tile_pool(name="wpool", bufs=1) as wp,
                tc.tile_pool(name="s1", bufs=2) as s1,
                tc.tile_pool(name="ps1", bufs=4, space="PSUM") as pp1,
            ):

                def load_x(tt):
                    # two half-loads + half-transposes for finer-grained deps
                    halves = []
                    for h in range(2):
                        x_ld = s1.tile([128, D_IN // 2], bf16, tag=f"xld{h}")
                        nc.gpsimd.dma_start(
                            out=x_ld[:],
                            in_=x_in.ap()[
                                tt * 128 : (tt + 1) * 128,
                                h * (D_IN // 2) : (h + 1) * (D_IN // 2),
                            ],
                        )
                        xT = s1.tile([128, KT1 // 2, 128], bf16, tag=f"xT{h}")
                        nc.sync.dma_start_transpose(xT[:], x_ld[:])
                        halves.append(xT)
                    return halves

                # first x tile ahead of the 16MB weight load (SWDGE is FIFO)
                xT_pre = {0: load_x(0)}

                w_sub = []
                for dh in range(2):
                    for g in range(4):
                        wt = wp.tile([128, 8, 512], bf16, tag=f"w_{dh}_{g}")
                        nc.gpsimd.dma_start(
                            out=wt[:],
                            in_=w_in.ap()[
                                :, dh * 512 : (dh + 1) * 512
                            ].rearrange("(kg p) n -> p kg n", p=128)[
                                :, g * 8 : (g + 1) * 8, :
                            ],
                        )
                        w_sub.append(wt)
                    if dh == 0:
                        xT_pre[1] = load_x(1)

                def w_tile(dh, kt):
                    return w_sub[dh * 4 + kt // 8][:, kt % 8, :]

                for tt in range(TT1):
                    xT = xT_pre.pop(tt) if tt in xT_pre else load_x(tt)
                    y_sb = s1.tile([128, D_OUT], bf16, tag="ysb")
                    for dh in range(2):
                        ps = pp1.tile([128, 512], f32)
                        dsl = slice(dh * 512, (dh + 1) * 512)
                        for kt in range(KT1):
                            nc.tensor.matmul(
                                ps[:],
                                lhsT=xT[kt // (KT1 // 2)][:, kt % (KT1 // 2), :],
                                rhs=w_tile(dh, kt),
                                start=(kt == 0),
                                stop=False,
                            )
                        # bias via rank-1 update: ones[1,128].T @ b[1,512]
                        nc.tensor.matmul(
                            ps[:],
                            lhsT=ones_sb[:1, :],
                            rhs=bias_sb[:1, dsl],
                            start=False,
                            stop=True,
                        )
                        nc.vector.tensor_copy(out=y_sb[:, dsl], in_=ps[:])
                    j = tt // 8
                    nc.sync.dma_start(
                        out=y_dram[j][(tt % 8) * 128 : (tt % 8 + 1) * 128, :],
                        in_=y_sb[:],
                    )
                    if tt == 7 or tt == 15:
                        nc.gpsimd.collective_compute(
                            "AllGather",
                            mybir.AluOpType.bypass,
                            replica_groups=[list(range(N_CORES))],
                            ins=[y_dram[j].ap().opt()],
                            outs=[flat_dram[j].ap().opt()],
                        )
                        load_chunk(0 if tt == 7 else 1)

            # ---------------- stage 2 ----------------
            with (
                tc.tile_pool(name="cwtp", bufs=4) as cwtp,
                tc.tile_pool(name="accp", bufs=1) as accp,
                tc.tile_pool(name="ps2", bufs=4, space="PSUM") as pp2,
            ):
                acc = accp.tile([128, TT2, D_OUT], f32)
                for hc in range(NHC):
                    if hc + 2 < NHC and (hc + 2) not in pend:
                        load_chunk(hc + 2)
                    flat_sb, cw_big = pend.pop(hc)
                    for t2 in range(TT2):
                        cwT = cwtp.tile([128, KT2, 128], bf16, tag="cwT")
                        nc.sync.dma_start_transpose(cwT[:], cw_big[:, t2, :])
                        for dh in range(2):
                            ps2 = pp2.tile([128, 512], f32)
                            dsl = slice(dh * 512, (dh + 1) * 512)
                            for kt in range(KT2):
                                nc.tensor.matmul(
                                    ps2[:],
                                    lhsT=cwT[:, kt, :],
                                    rhs=flat_sb[:, kt, dsl],
                                    start=(kt == 0),
                                    stop=(kt == KT2 - 1),
                                )
                            if hc == 0:
                                nc.vector.tensor_copy(
                                    out=acc[:, t2, dsl], in_=ps2[:]
                                )
                            else:
                                nc.vector.tensor_add(
                                    acc[:, t2, dsl], acc[:, t2, dsl], ps2[:]
                                )
                        if hc == NHC - 1:
                            nc.sync.dma_start(
                                out=out_t.ap()[t2 * 128 : (t2 + 1) * 128, :],
                                in_=acc[:, t2, :],
                            )

    nc.compile()
    return nc


def _get_nc():
    if "nc" not in _CACHE:
        _CACHE["nc"] = _build_nc()
    return _CACHE["nc"]


def make_in_maps(inputs, combine_weights, weight, bias):
    inputs = np.ascontiguousarray(np.asarray(inputs, dtype=np.float32))
    combine_weights = np.ascontiguousarray(
        np.asarray(combine_weights, dtype=np.float32)
    )
    weight = np.ascontiguousarray(np.asarray(weight, dtype=np.float32))
    bias = np.ascontiguousarray(np.asarray(bias, dtype=np.float32))

    in_maps = []
    for c in range(N_CORES):
        in_maps.append(
            {
                "x": inputs[c * CAP : (c + 1) * CAP],
                "w": weight[c],
                "b": bias[c].reshape(1, D_OUT),
                "cw": combine_weights[c * TOK : (c + 1) * TOK].reshape(
                    TOK, E * CAP
                ),
            }
        )
    return in_maps


def kernel(inputs, combine_weights, weight, bias):
    from concourse.bass_utils import run_bass_kernel_spmd

    nc = _get_nc()
    in_maps = make_in_maps(inputs, combine_weights, weight, bias)
    res = run_bass_kernel_spmd(nc, in_maps, core_ids=list(range(N_CORES)))
    _CACHE["last_results"] = res

    out = np.concatenate([r["out"] for r in res.results], axis=0)
    return out.reshape(4, 2048, D_OUT).astype(np.float32, copy=False)
